# revision 33
# baseline (speedup 1.0000x reference)
"""Trainium2 Bass kernel for nn_BertOutput (binary-quantized BERT output layer).

Computation (see reference):
    w_scale = mean(|W|, axis=1)                  # [H, 1]
    W_q     = w_scale * sign(W)                  # [H, I]
    x_q     = clip * sign(x / clip)              # [B, S, I]
    h       = x_q @ W_q.T + b                    # [B, S, H]
    out     = LayerNorm(h + input_tensor) * gamma + beta

Key structural facts exploited:
  * x_q @ W_q.T == (|clip| * w_scale[h]) * (sign(x) @ sign(W).T) -- the matmul
    operands are exactly +-1, representable exactly in fp8e4, and the K=4096
    accumulation of +-1 terms is exact in fp32 PSUM.  The per-output-channel
    scale is applied after the matmul in fp32.
  * fp8 +-1 operands allow the DoubleRow perf mode: each matmul contracts
    K=256 (2 fp8 weights per PE cell), halving the matmul count.
  * sign() survives fp32->bf16 round-to-nearest; x is cast to bf16 during the
    DMA load, sign is a 2-op bitwise trick on the DVE, then a copy to fp8.
    W is loaded as f32 (HWDGE) and signed on the scalar engine (SIGN
    activation with fp8 output).
  * The transposes (contraction dim to partitions) use the DMA xbar transpose
    on PAIRS of fp8 values viewed as uint16 -- half the xbar traffic of bf16.
    The resulting (value, k-parity j) interleave is consumed by stride-2
    slices in the matmul APs.

Hard-won HW constraints honoured here (found via sim-vs-HW divergences):
  * ALL xbar transposes are issued from the Sync (SP) HWDGE ring.  Concurrent
    DMA transposes on the two HWDGE rings (SP + ACT) corrupt each other
    (every-16th-element survivor patterns).
  * No in-place DVE ops (out aliasing in0) -- wrong results on HW.
  * DMA issues must not queue behind data-waiting compute on the same engine
    FIFO (head-of-line blocking collapses DMA concurrency): W loads are
    emitted 8 ahead of the signs that consume them.

Sharding: plain data-parallel over tokens -- 8192 tokens -> 1024 per core.
Each core computes a full LayerNorm over hidden=1024, so no collectives.
"""

import sys

sys.path.insert(0, "/opt/trn_rl_repo")

import numpy as np

import concourse.bass as bass  # noqa: F401  (import side effects / registry)
import concourse.tile as tile
from concourse import bacc, bass_utils, mybir

F32 = mybir.dt.float32
BF16 = mybir.dt.bfloat16
F8 = mybir.dt.float8e4
U16 = mybir.dt.uint16

HIDDEN = 1024
INTER = 4096
TOKENS = 8192
N_CORES = 8
TPC = TOKENS // N_CORES          # tokens per core = 1024
M_TILES = TPC // 128             # 8 token tiles per core
K_BLOCKS = INTER // 128          # 32 contraction blocks
NQ = 4                           # W/k-space processed in 4 quarters
KQ = K_BLOCKS // NQ              # 8 k-blocks per quarter
H_TILES = HIDDEN // 128          # 8 row tiles of W
NB = 16                          # u16-packed k blocks (256 k each)
EPS = 1e-12

TRACE = False                    # set True from test harness to profile
TRACE_ALL_CORES = False

_cache: dict = {}


def _install_ntff_hook():
    """The agent image's antenv package lacks axon_hooks, which silently
    disables NTFF profiling under axon.  Recreate it and wire the ctypes
    hook from trn_agent_boot (profiling/tooling only; the compute path
    does not depend on this)."""
    import types

    import antenv
    if getattr(antenv, "axon_hooks", None) is not None:
        return
    from trn_agent_boot.trn_boot import _ntff_profile_via_ctypes
    mod = types.ModuleType("antenv.axon_hooks")
    mod._hook = _ntff_profile_via_ctypes("/opt/axon/libaxon_pjrt.so")
    mod.get_axon_ntff_profile_hook = lambda: mod._hook

    def _set(h):
        mod._hook = h
    mod.set_axon_ntff_profile_hook = _set
    sys.modules["antenv.axon_hooks"] = mod
    antenv.axon_hooks = mod


def _build(scale_mul: float, use_b: bool, use_gamma: bool, use_beta: bool):
    """Build the SPMD program (identical on all 8 cores).

    scale_mul = |clip_val| / INTER, folded into the per-channel scale.
    """
    nc = bacc.Bacc("TRN2", target_bir_lowering=False, debug=False,
                   num_devices=N_CORES)

    x_ap = nc.dram_tensor("x", [TPC, INTER], F32, kind="ExternalInput").ap()
    res_ap = nc.dram_tensor("res", [TPC, HIDDEN], F32, kind="ExternalInput").ap()
    w_ap = nc.dram_tensor("w", [HIDDEN, INTER], F32, kind="ExternalInput").ap()
    b_ap = g_ap = be_ap = None
    if use_b:
        b_ap = nc.dram_tensor("bvec", [HIDDEN], F32, kind="ExternalInput").ap()
    if use_gamma:
        g_ap = nc.dram_tensor("gvec", [HIDDEN], F32, kind="ExternalInput").ap()
    if use_beta:
        be_ap = nc.dram_tensor("bevec", [HIDDEN], F32, kind="ExternalInput").ap()
    y_ap = nc.dram_tensor("y", [TPC, HIDDEN], F32, kind="ExternalOutput").ap()

    AT = mybir.AluOpType
    AF = mybir.ActivationFunctionType
    DR = mybir.MatmulPerfMode.DoubleRow

    epi_bufs = 1 if (use_b or use_gamma or use_beta) else 2

    with tile.TileContext(nc) as tc:
        with (
            tc.tile_pool(name="wt", bufs=1) as wt_pool,
            tc.tile_pool(name="wq", bufs=8) as wq_pool,
            tc.tile_pool(name="wrest", bufs=4) as wrest_pool,
            tc.tile_pool(name="wsg", bufs=4) as wsg_pool,
            tc.tile_pool(name="const", bufs=1) as const_pool,
            tc.tile_pool(name="xio", bufs=2) as xio_pool,
            tc.tile_pool(name="x8", bufs=1) as x8_pool,
            tc.tile_pool(name="yo", bufs=1) as yo_pool,
            tc.tile_pool(name="xt", bufs=1) as xt_pool,
            tc.tile_pool(name="inp", bufs=2) as inp_pool,
            tc.tile_pool(name="epi", bufs=epi_bufs) as epi_pool,
            tc.tile_pool(name="stats", bufs=2) as stats_pool,
            tc.tile_pool(name="psum", bufs=4, space="PSUM") as psum_pool,
            tc.tile_pool(name="dram", bufs=1, space="DRAM") as dram_pool,
        ):
            # ---------------- constants ----------------
            epsT = const_pool.tile([128, 1], F32, tag="epsT")
            nc.vector.memset(epsT[:], float(EPS))

            bB = gB = beB = None
            if use_b:
                bB = const_pool.tile([128, HIDDEN], F32, tag="bB")
                nc.sync.dma_start(
                    bB[:],
                    b_ap.rearrange("(a n) -> a n", a=1).broadcast_to([128, HIDDEN]))
            if use_gamma:
                gB = const_pool.tile([128, HIDDEN], F32, tag="gB")
                nc.sync.dma_start(
                    gB[:],
                    g_ap.rearrange("(a n) -> a n", a=1).broadcast_to([128, HIDDEN]))
            if use_beta:
                beB = const_pool.tile([128, HIDDEN], F32, tag="beB")
                nc.sync.dma_start(
                    beB[:],
                    be_ap.rearrange("(a n) -> a n", a=1).broadcast_to([128, HIDDEN]))

            # ---------------- W / x preparation ----------------
            # wT8u[n][q][p, b, (ht%4)*128 + c] (u16) packs the fp8 pair
            #   sign(W[h, k]), sign(W[h, k+1])
            # for h = n*512 + within, k = q*1024 + 2*(b*128 + p).
            wT8u = [[wt_pool.tile([128, NQ, 512], U16, tag=f"wT{n}{q}",
                                  name=f"wT{n}{q}") for q in range(NQ)]
                    for n in range(2)]
            # per-ht partial row-sums of |W|: A = quarter 0, B = quarters 1-3
            wsumA = const_pool.tile([128, H_TILES], F32, tag="wsumA")
            wsumB = const_pool.tile([128, H_TILES], F32, tag="wsumB")

            wq0s, wrests = {}, {}

            def w_load_q0(ht):
                # quarter 0 as small chunks for the fastest matmul start
                wq = wq_pool.tile([128, 1024], F32, tag="wq")
                # NOTE: keep plain loads OFF the Sync ring -- mixing them
                # with the xbar transposes there wedged the device
                # (NRT_EXEC_UNIT_UNRECOVERABLE).
                nc.scalar.dma_start(
                    wq[:], w_ap[ht * 128:(ht + 1) * 128, 0:1024])
                wq0s[ht] = wq

            def w_load_rest(ht):
                # quarters 1-3 as wide row loads (12KB descriptors)
                wr = wrest_pool.tile([128, 3072], F32, tag="wrest")
                nc.scalar.dma_start(
                    wr[:], w_ap[ht * 128:(ht + 1) * 128, 1024:4096])
                wrests[ht] = wr

            def w_sign_transpose(src_ap, q, ht):
                wsg8 = wsg_pool.tile([128, 1024], F8, tag="wsg8")
                nc.scalar.sign(wsg8[:], src_ap)
                # NOTE: ALL xbar transposes are issued from the Sync (SP)
                # HWDGE ring.  Concurrent DMA transposes on the two HWDGE
                # rings (SP + ACT) corrupt each other on HW.
                nc.sync.dma_start(
                    out=wT8u[ht // 4][q][:, :,
                                         (ht % 4) * 128:(ht % 4 + 1) * 128],
                    in_=wsg8[:].bitcast(U16), transpose=True)

            def w_process_q0(ht):
                wq = wq0s.pop(ht)
                nc.vector.tensor_reduce(
                    wsumA[:, ht:ht + 1], wq[:],
                    axis=mybir.AxisListType.X, op=AT.add,
                    apply_absolute_value=True)
                w_sign_transpose(wq[:], 0, ht)

            def w_process_rest(ht):
                wr = wrests.pop(ht)
                nc.vector.tensor_reduce(
                    wsumB[:, ht:ht + 1], wr[:],
                    axis=mybir.AxisListType.X, op=AT.add,
                    apply_absolute_value=True)
                for q in range(1, NQ):
                    w_sign_transpose(wr[:, (q - 1) * 1024:q * 1024], q, ht)

            xvs = {}

            def prep_x_tile(m):
                xin = xio_pool.tile([128, INTER], BF16, tag="xin")
                nc.gpsimd.dma_start(xin[:], x_ap[m * 128:(m + 1) * 128, :])
                xsg = xio_pool.tile([128, INTER], BF16, tag="xsg")
                # sign via bit trick: (v & 0x8000) | 0x3F80 -> +-1.0 bf16
                nc.vector.tensor_scalar(
                    out=xsg[:].bitcast(U16), in0=xin[:].bitcast(U16),
                    scalar1=0x8000, scalar2=0x3F80,
                    op0=AT.bitwise_and, op1=AT.bitwise_or)
                xsg8 = x8_pool.tile([128, INTER], F8, tag="xsg8")
                nc.vector.tensor_copy(xsg8[:], xsg[:])
                # xT8u[m][p, b, t] (u16) packs the fp8 pair
                #   sign(x[t, k]), sign(x[t, k+1])   for k = 2*(b*128 + p)
                xT8u = xt_pool.tile([128, NB, 128], U16, tag=f"xT{m % 5}")
                nc.sync.dma_start(out=xT8u[:], in_=xsg8[:].bitcast(U16),
                                  transpose=True)
                xvs[m] = xT8u[:].bitcast(F8).rearrange(
                    "p b (t j) -> p b t j", j=2)

            # emission order: x loads + W loads lead their consumers so DMA
            # issues never starve behind data-waiting compute on the FIFOs.
            # q0's loads go out first with only x0 competing for DMA; the
            # rest of W streams as 8 wide row loads; later x tiles are
            # spaced between batches.
            for ht in range(H_TILES):
                w_load_q0(ht)
            prep_x_tile(0)
            w_load_rest(0)
            w_load_rest(1)
            prep_x_tile(1)
            w_load_rest(2)
            for ht in range(H_TILES):    # q0 processed, rest rows stream in
                w_process_q0(ht)
                if ht + 3 < H_TILES:
                    w_load_rest(ht + 3)
            prep_x_tile(2)
            w_process_rest(0)
            prep_x_tile(3)
            w_process_rest(1)
            w_process_rest(2)
            prep_x_tile(4)
            w_process_rest(3)
            w_process_rest(4)
            prep_x_tile(5)
            w_process_rest(5)
            prep_x_tile(6)
            w_process_rest(6)
            w_process_rest(7)
            prep_x_tile(7)

            # ---------------- per-channel scale ----------------
            # scale[h] = |clip|/INTER * sum_i |W[h, i]|
            scale8p = const_pool.tile([128, H_TILES], F32, tag="scale8p")
            nc.vector.tensor_add(scale8p[:], wsumA[:], wsumB[:])
            scale8 = const_pool.tile([128, H_TILES], F32, tag="scale8")
            nc.vector.tensor_scalar(out=scale8[:], in0=scale8p[:],
                                    scalar1=float(scale_mul), scalar2=None,
                                    op0=AT.mult)
            scratch = dram_pool.tile([HIDDEN], F32)
            # h = j*128 + p  ->  dram[j*128+p] = scale8[p, j]
            nc.gpsimd.dma_start(
                out=scratch[:].rearrange("(j p) -> p j", p=128), in_=scale8[:])
            scaleF = const_pool.tile([128, HIDDEN], F32, tag="scaleF")
            nc.sync.dma_start(
                scaleF[:],
                scratch[:].rearrange("(a n) -> a n", a=1).broadcast_to([128, HIDDEN]))

            wvs = [[wT8u[n][q][:].bitcast(F8).rearrange(
                        "p b (h j) -> p b h j", j=2)
                    for q in range(NQ)] for n in range(2)]

            # ---------------- matmul + epilogue, two groups of 4 ------------
            def epilogue(m, psum, inp):
                # r = psum * scaleF + inp (+ bB), then LayerNorm
                t = epi_pool.tile([128, HIDDEN], F32, tag="t")
                nc.vector.tensor_mul(t[:], psum[:], scaleF[:])
                r = epi_pool.tile([128, HIDDEN], F32, tag="r")
                nc.vector.tensor_add(r[:], t[:], inp[:])
                if use_b:
                    r2 = epi_pool.tile([128, HIDDEN], F32, tag="r2")
                    nc.vector.tensor_add(r2[:], r[:], bB[:])
                    r = r2

                bn6 = stats_pool.tile([128, 2, 6], F32, tag="bn6")
                nc.vector.bn_stats(bn6[:, 0, :], r[:, 0:512])
                nc.vector.bn_stats(bn6[:, 1, :], r[:, 512:1024])
                mv = stats_pool.tile([128, 2], F32, tag="mv")
                nc.vector.bn_aggr(mv[:], bn6[:])
                sd = stats_pool.tile([128, 1], F32, tag="sd")
                nc.scalar.activation(sd[:], mv[:, 1:2], AF.Sqrt,
                                     bias=epsT[:, 0:1])
                rstd = stats_pool.tile([128, 1], F32, tag="rstd")
                nc.vector.reciprocal(rstd[:], sd[:])
                nm = stats_pool.tile([128, 1], F32, tag="nm")
                nc.vector.tensor_scalar(out=nm[:], in0=mv[:, 0:1],
                                        scalar1=rstd[:, 0:1], scalar2=-1.0,
                                        op0=AT.mult, op1=AT.mult)
                y = yo_pool.tile([128, HIDDEN], F32, tag="y")
                nc.scalar.activation(y[:], r[:], AF.Identity,
                                     bias=nm[:, 0:1], scale=rstd[:, 0:1])
                if use_gamma:
                    y2 = epi_pool.tile([128, HIDDEN], F32, tag="y2")
                    nc.vector.tensor_mul(y2[:], y[:], gB[:])
                    y = y2
                if use_beta:
                    y3 = epi_pool.tile([128, HIDDEN], F32, tag="y3")
                    nc.vector.tensor_add(y3[:], y[:], beB[:])
                    y = y3

                nc.gpsimd.dma_start(y_ap[m * 128:(m + 1) * 128, :], y[:])

            for group in ((0, 1, 2, 3), (4, 5, 6, 7)):
                psums, inps = {}, {}
                for m in group:
                    inp = inp_pool.tile([128, HIDDEN], F32, tag="inp")
                    nc.scalar.dma_start(inp[:], res_ap[m * 128:(m + 1) * 128, :])
                    inps[m] = inp
                    psums[m] = psum_pool.tile([128, HIDDEN], F32, tag="ps",
                                              name="ps")
                for q in range(NQ):
                    for m in group:
                        for n in range(2):
                            for bp in range(2):
                                for j in range(2):
                                    B0 = 4 * q + 2 * bp
                                    nc.tensor.matmul(
                                        psums[m][:, n * 512:(n + 1) * 512],
                                        lhsT=xvs[m][:, B0:B0 + 2, :, j:j + 1],
                                        rhs=wvs[n][q][:, 2 * bp:2 * bp + 2,
                                                      :, j:j + 1],
                                        start=(q == 0 and bp == 0 and j == 0),
                                        stop=(q == NQ - 1 and bp == 1
                                              and j == 1),
                                        perf_mode=DR)
                for m in group:
                    epilogue(m, psums[m], inps[m])

    nc.compile()
    return nc


_last_results = None


def kernel(hidden_states, input_tensor, W, b, clip_val, gamma, beta):
    global _last_results
    hidden_states = np.asarray(hidden_states)
    input_tensor = np.asarray(input_tensor)
    W = np.asarray(W, dtype=np.float32)
    b = np.asarray(b, dtype=np.float32)
    gamma = np.asarray(gamma, dtype=np.float32)
    beta = np.asarray(beta, dtype=np.float32)
    clip = float(np.asarray(clip_val))

    use_b = bool(np.any(b != 0.0))
    use_gamma = bool(np.any(gamma != 1.0))
    use_beta = bool(np.any(beta != 0.0))
    scale_mul = abs(clip) / INTER

    key = (scale_mul, use_b, use_gamma, use_beta)
    if key not in _cache:
        _cache[key] = _build(scale_mul, use_b, use_gamma, use_beta)
    nc = _cache[key]

    hs = np.ascontiguousarray(
        hidden_states.reshape(TOKENS, INTER).astype(np.float32, copy=False))
    rs = np.ascontiguousarray(
        input_tensor.reshape(TOKENS, HIDDEN).astype(np.float32, copy=False))
    Wc = np.ascontiguousarray(W)

    in_maps = []
    for c in range(N_CORES):
        m = {
            "x": np.ascontiguousarray(hs[c * TPC:(c + 1) * TPC]),
            "res": np.ascontiguousarray(rs[c * TPC:(c + 1) * TPC]),
            "w": Wc,
        }
        if use_b:
            m["bvec"] = b
        if use_gamma:
            m["gvec"] = gamma
        if use_beta:
            m["bevec"] = beta
        in_maps.append(m)

    kwargs = {}
    if TRACE:
        _install_ntff_hook()
        kwargs["trace"] = True
        if TRACE_ALL_CORES:
            kwargs["trace_cores"] = list(range(N_CORES))
    res = bass_utils.run_bass_kernel_spmd(
        nc, in_maps, core_ids=list(range(N_CORES)), **kwargs)
    _last_results = res

    y = np.concatenate([res.results[c]["y"] for c in range(N_CORES)], axis=0)
    return y.reshape(hidden_states.shape[:-1] + (HIDDEN,)).astype(np.float32)


# revision 35
# speedup vs baseline: 1.0414x; 1.0414x over previous
"""Trainium2 Bass kernel for nn_BertOutput (binary-quantized BERT output layer).

Computation (see reference):
    w_scale = mean(|W|, axis=1)                  # [H, 1]
    W_q     = w_scale * sign(W)                  # [H, I]
    x_q     = clip * sign(x / clip)              # [B, S, I]
    h       = x_q @ W_q.T + b                    # [B, S, H]
    out     = LayerNorm(h + input_tensor) * gamma + beta

Key structural facts exploited:
  * x_q @ W_q.T == (|clip| * w_scale[h]) * (sign(x) @ sign(W).T) -- the matmul
    operands are exactly +-1, representable exactly in fp8e4, and the K=4096
    accumulation of +-1 terms is exact in fp32 PSUM.  The per-output-channel
    scale is applied after the matmul in fp32.
  * fp8 +-1 operands allow the DoubleRow perf mode: each matmul contracts
    K=256 (2 fp8 weights per PE cell), halving the matmul count.
  * sign() survives fp32->bf16 round-to-nearest; x is cast to bf16 during the
    DMA load, sign is a 2-op bitwise trick on the DVE, then a copy to fp8.
    W is loaded as f32 (HWDGE) and signed on the scalar engine (SIGN
    activation with fp8 output).
  * The transposes (contraction dim to partitions) use the DMA xbar transpose
    on PAIRS of fp8 values viewed as uint16 -- half the xbar traffic of bf16.
    The resulting (value, k-parity j) interleave is consumed by stride-2
    slices in the matmul APs.

Hard-won HW constraints honoured here (found via sim-vs-HW divergences):
  * ALL xbar transposes are issued from the Sync (SP) HWDGE ring.  Concurrent
    DMA transposes on the two HWDGE rings (SP + ACT) corrupt each other
    (every-16th-element survivor patterns).
  * No in-place DVE ops (out aliasing in0) -- wrong results on HW.
  * DMA issues must not queue behind data-waiting compute on the same engine
    FIFO (head-of-line blocking collapses DMA concurrency): W loads are
    emitted 8 ahead of the signs that consume them.

Sharding: plain data-parallel over tokens -- 8192 tokens -> 1024 per core.
Each core computes a full LayerNorm over hidden=1024, so no collectives.
"""

import sys

sys.path.insert(0, "/opt/trn_rl_repo")

import numpy as np

import concourse.bass as bass  # noqa: F401  (import side effects / registry)
import concourse.tile as tile
from concourse import bacc, bass_utils, mybir

F32 = mybir.dt.float32
BF16 = mybir.dt.bfloat16
F8 = mybir.dt.float8e4
U16 = mybir.dt.uint16

HIDDEN = 1024
INTER = 4096
TOKENS = 8192
N_CORES = 8
TPC = TOKENS // N_CORES          # tokens per core = 1024
M_TILES = TPC // 128             # 8 token tiles per core
K_BLOCKS = INTER // 128          # 32 contraction blocks
NQ = 4                           # W/k-space processed in 4 quarters
KQ = K_BLOCKS // NQ              # 8 k-blocks per quarter
H_TILES = HIDDEN // 128          # 8 row tiles of W
NB = 16                          # u16-packed k blocks (256 k each)
EPS = 1e-12

TRACE = False                    # set True from test harness to profile
TRACE_ALL_CORES = False

_cache: dict = {}


def _install_ntff_hook():
    """The agent image's antenv package lacks axon_hooks, which silently
    disables NTFF profiling under axon.  Recreate it and wire the ctypes
    hook from trn_agent_boot (profiling/tooling only; the compute path
    does not depend on this)."""
    import types

    import antenv
    if getattr(antenv, "axon_hooks", None) is not None:
        return
    from trn_agent_boot.trn_boot import _ntff_profile_via_ctypes
    mod = types.ModuleType("antenv.axon_hooks")
    mod._hook = _ntff_profile_via_ctypes("/opt/axon/libaxon_pjrt.so")
    mod.get_axon_ntff_profile_hook = lambda: mod._hook

    def _set(h):
        mod._hook = h
    mod.set_axon_ntff_profile_hook = _set
    sys.modules["antenv.axon_hooks"] = mod
    antenv.axon_hooks = mod


def _build(scale_mul: float, use_b: bool, use_gamma: bool, use_beta: bool):
    """Build the SPMD program (identical on all 8 cores).

    scale_mul = |clip_val| / INTER, folded into the per-channel scale.
    """
    nc = bacc.Bacc("TRN2", target_bir_lowering=False, debug=False,
                   num_devices=N_CORES)

    x_ap = nc.dram_tensor("x", [TPC, INTER], F32, kind="ExternalInput").ap()
    res_ap = nc.dram_tensor("res", [TPC, HIDDEN], F32, kind="ExternalInput").ap()
    w_ap = nc.dram_tensor("w", [HIDDEN, INTER], F32, kind="ExternalInput").ap()
    b_ap = g_ap = be_ap = None
    if use_b:
        b_ap = nc.dram_tensor("bvec", [HIDDEN], F32, kind="ExternalInput").ap()
    if use_gamma:
        g_ap = nc.dram_tensor("gvec", [HIDDEN], F32, kind="ExternalInput").ap()
    if use_beta:
        be_ap = nc.dram_tensor("bevec", [HIDDEN], F32, kind="ExternalInput").ap()
    y_ap = nc.dram_tensor("y", [TPC, HIDDEN], F32, kind="ExternalOutput").ap()

    AT = mybir.AluOpType
    AF = mybir.ActivationFunctionType
    DR = mybir.MatmulPerfMode.DoubleRow

    epi_bufs = 1 if (use_b or use_gamma or use_beta) else 2

    with tile.TileContext(nc) as tc:
        with (
            tc.tile_pool(name="wt", bufs=1) as wt_pool,
            tc.tile_pool(name="wq", bufs=8) as wq_pool,
            tc.tile_pool(name="wrest", bufs=3) as wrest_pool,
            tc.tile_pool(name="wsg", bufs=4) as wsg_pool,
            tc.tile_pool(name="const", bufs=1) as const_pool,
            tc.tile_pool(name="xio", bufs=2) as xio_pool,
            tc.tile_pool(name="xt", bufs=1) as xt_pool,
            tc.tile_pool(name="inp", bufs=2) as inp_pool,
            tc.tile_pool(name="epi", bufs=epi_bufs) as epi_pool,
            tc.tile_pool(name="stats", bufs=2) as stats_pool,
            tc.tile_pool(name="psum", bufs=4, space="PSUM") as psum_pool,
            tc.tile_pool(name="dram", bufs=1, space="DRAM") as dram_pool,
        ):
            # ---------------- constants ----------------
            epsT = const_pool.tile([128, 1], F32, tag="epsT")
            nc.vector.memset(epsT[:], float(EPS))

            bB = gB = beB = None
            if use_b:
                bB = const_pool.tile([128, HIDDEN], F32, tag="bB")
                nc.sync.dma_start(
                    bB[:],
                    b_ap.rearrange("(a n) -> a n", a=1).broadcast_to([128, HIDDEN]))
            if use_gamma:
                gB = const_pool.tile([128, HIDDEN], F32, tag="gB")
                nc.sync.dma_start(
                    gB[:],
                    g_ap.rearrange("(a n) -> a n", a=1).broadcast_to([128, HIDDEN]))
            if use_beta:
                beB = const_pool.tile([128, HIDDEN], F32, tag="beB")
                nc.sync.dma_start(
                    beB[:],
                    be_ap.rearrange("(a n) -> a n", a=1).broadcast_to([128, HIDDEN]))

            # ---------------- W / x preparation ----------------
            # wT8u[n][q][p, b, (ht%4)*128 + c] (u16) packs the fp8 pair
            #   sign(W[h, k]), sign(W[h, k+1])
            # for h = n*512 + within, k = q*1024 + 2*(b*128 + p).
            wT8u = [[wt_pool.tile([128, NQ, 512], U16, tag=f"wT{n}{q}",
                                  name=f"wT{n}{q}") for q in range(NQ)]
                    for n in range(2)]
            # per-ht partial row-sums of |W|: A = quarter 0, B = quarters 1-3
            wsumA = const_pool.tile([128, H_TILES], F32, tag="wsumA")
            wsumB = const_pool.tile([128, H_TILES], F32, tag="wsumB")

            wq0s, wrests = {}, {}

            def w_load_q0(ht):
                # quarter 0 as small chunks for the fastest matmul start
                wq = wq_pool.tile([128, 1024], F32, tag="wq")
                # NOTE: keep plain loads OFF the Sync ring -- mixing them
                # with the xbar transposes there wedged the device
                # (NRT_EXEC_UNIT_UNRECOVERABLE).
                nc.scalar.dma_start(
                    wq[:], w_ap[ht * 128:(ht + 1) * 128, 0:1024])
                wq0s[ht] = wq

            def w_load_rest(ht):
                # quarters 1-3 as wide row loads (12KB descriptors)
                wr = wrest_pool.tile([128, 3072], F32, tag="wrest")
                nc.scalar.dma_start(
                    wr[:], w_ap[ht * 128:(ht + 1) * 128, 1024:4096])
                wrests[ht] = wr

            def w_sign_transpose(src_ap, q, ht):
                wsg8 = wsg_pool.tile([128, 1024], F8, tag="wsg8")
                nc.scalar.sign(wsg8[:], src_ap)
                # NOTE: ALL xbar transposes are issued from the Sync (SP)
                # HWDGE ring.  Concurrent DMA transposes on the two HWDGE
                # rings (SP + ACT) corrupt each other on HW.
                nc.sync.dma_start(
                    out=wT8u[ht // 4][q][:, :,
                                         (ht % 4) * 128:(ht % 4 + 1) * 128],
                    in_=wsg8[:].bitcast(U16), transpose=True)

            def w_process_q0(ht):
                wq = wq0s.pop(ht)
                nc.vector.tensor_reduce(
                    wsumA[:, ht:ht + 1], wq[:],
                    axis=mybir.AxisListType.X, op=AT.add,
                    apply_absolute_value=True)
                w_sign_transpose(wq[:], 0, ht)

            def w_process_rest(ht):
                wr = wrests.pop(ht)
                nc.vector.tensor_reduce(
                    wsumB[:, ht:ht + 1], wr[:],
                    axis=mybir.AxisListType.X, op=AT.add,
                    apply_absolute_value=True)
                for q in range(1, NQ):
                    w_sign_transpose(wr[:, (q - 1) * 1024:q * 1024], q, ht)

            xvs = {}

            def prep_x_tile(m):
                xin = xio_pool.tile([128, INTER], BF16, tag="xin")
                nc.gpsimd.dma_start(xin[:], x_ap[m * 128:(m + 1) * 128, :])
                xsg = xio_pool.tile([128, INTER], BF16, tag="xsg")
                # sign via bit trick: (v & 0x8000) | 0x3F80 -> +-1.0 bf16
                nc.vector.tensor_scalar(
                    out=xsg[:].bitcast(U16), in0=xin[:].bitcast(U16),
                    scalar1=0x8000, scalar2=0x3F80,
                    op0=AT.bitwise_and, op1=AT.bitwise_or)
                xsg8 = xio_pool.tile([128, INTER], F8, tag="xsg8")
                nc.vector.tensor_copy(xsg8[:], xsg[:])
                # xT8u[m][p, b, t] (u16) packs the fp8 pair
                #   sign(x[t, k]), sign(x[t, k+1])   for k = 2*(b*128 + p)
                xT8u = xt_pool.tile([128, NB, 128], U16, tag=f"xT{m % 6}")
                nc.sync.dma_start(out=xT8u[:], in_=xsg8[:].bitcast(U16),
                                  transpose=True)
                xvs[m] = xT8u[:].bitcast(F8).rearrange(
                    "p b (t j) -> p b t j", j=2)

            # emission order: x loads + W loads lead their consumers so DMA
            # issues never starve behind data-waiting compute on the FIFOs.
            # q0's loads go out first with only x0 competing for DMA; the
            # rest of W streams as 8 wide row loads; later x tiles are
            # spaced between batches.
            # q0's batch gets the early DMA window alone (plus x0/x1);
            # rest-row loads issue behind the first q0 signs so their
            # transfers don't flatten q0's delivery.
            for ht in range(H_TILES):
                w_load_q0(ht)
            prep_x_tile(0)
            prep_x_tile(1)
            for ht in range(H_TILES):    # q0 processed, rest rows ramp up
                w_process_q0(ht)
                if ht < 3:
                    w_load_rest(ht)
            prep_x_tile(2)
            w_process_rest(0)
            w_load_rest(3)
            prep_x_tile(3)
            w_process_rest(1)
            w_load_rest(4)
            w_process_rest(2)
            w_load_rest(5)
            prep_x_tile(4)
            w_process_rest(3)
            w_load_rest(6)
            w_process_rest(4)
            w_load_rest(7)
            prep_x_tile(5)
            w_process_rest(5)
            prep_x_tile(6)
            w_process_rest(6)
            w_process_rest(7)
            prep_x_tile(7)

            # ---------------- per-channel scale ----------------
            # scale[h] = |clip|/INTER * sum_i |W[h, i]|
            scale8p = const_pool.tile([128, H_TILES], F32, tag="scale8p")
            nc.vector.tensor_add(scale8p[:], wsumA[:], wsumB[:])
            scale8 = const_pool.tile([128, H_TILES], F32, tag="scale8")
            nc.vector.tensor_scalar(out=scale8[:], in0=scale8p[:],
                                    scalar1=float(scale_mul), scalar2=None,
                                    op0=AT.mult)
            scratch = dram_pool.tile([HIDDEN], F32)
            # h = j*128 + p  ->  dram[j*128+p] = scale8[p, j]
            nc.gpsimd.dma_start(
                out=scratch[:].rearrange("(j p) -> p j", p=128), in_=scale8[:])
            scaleF = const_pool.tile([128, HIDDEN], F32, tag="scaleF")
            nc.sync.dma_start(
                scaleF[:],
                scratch[:].rearrange("(a n) -> a n", a=1).broadcast_to([128, HIDDEN]))

            wvs = [[wT8u[n][q][:].bitcast(F8).rearrange(
                        "p b (h j) -> p b h j", j=2)
                    for q in range(NQ)] for n in range(2)]

            # ---------------- matmul + epilogue, two groups of 4 ------------
            def epilogue(m, psum, inp):
                # r = psum * scaleF + inp (+ bB), then LayerNorm
                t = epi_pool.tile([128, HIDDEN], F32, tag="t")
                nc.vector.tensor_mul(t[:], psum[:], scaleF[:])
                r = epi_pool.tile([128, HIDDEN], F32, tag="r")
                nc.vector.tensor_add(r[:], t[:], inp[:])
                if use_b:
                    r2 = epi_pool.tile([128, HIDDEN], F32, tag="r2")
                    nc.vector.tensor_add(r2[:], r[:], bB[:])
                    r = r2

                bn6 = stats_pool.tile([128, 2, 6], F32, tag="bn6")
                nc.vector.bn_stats(bn6[:, 0, :], r[:, 0:512])
                nc.vector.bn_stats(bn6[:, 1, :], r[:, 512:1024])
                mv = stats_pool.tile([128, 2], F32, tag="mv")
                nc.vector.bn_aggr(mv[:], bn6[:])
                sd = stats_pool.tile([128, 1], F32, tag="sd")
                nc.scalar.activation(sd[:], mv[:, 1:2], AF.Sqrt,
                                     bias=epsT[:, 0:1])
                rstd = stats_pool.tile([128, 1], F32, tag="rstd")
                nc.vector.reciprocal(rstd[:], sd[:])
                nm = stats_pool.tile([128, 1], F32, tag="nm")
                nc.vector.tensor_scalar(out=nm[:], in0=mv[:, 0:1],
                                        scalar1=rstd[:, 0:1], scalar2=-1.0,
                                        op0=AT.mult, op1=AT.mult)
                y = epi_pool.tile([128, HIDDEN], F32, tag="y")
                nc.scalar.activation(y[:], r[:], AF.Identity,
                                     bias=nm[:, 0:1], scale=rstd[:, 0:1])
                if use_gamma:
                    y2 = epi_pool.tile([128, HIDDEN], F32, tag="y2")
                    nc.vector.tensor_mul(y2[:], y[:], gB[:])
                    y = y2
                if use_beta:
                    y3 = epi_pool.tile([128, HIDDEN], F32, tag="y3")
                    nc.vector.tensor_add(y3[:], y[:], beB[:])
                    y = y3

                nc.gpsimd.dma_start(y_ap[m * 128:(m + 1) * 128, :], y[:])

            for group in ((0, 1, 2, 3), (4, 5, 6, 7)):
                psums, inps = {}, {}
                for m in group:
                    inp = inp_pool.tile([128, HIDDEN], F32, tag="inp")
                    nc.scalar.dma_start(inp[:], res_ap[m * 128:(m + 1) * 128, :])
                    inps[m] = inp
                    psums[m] = psum_pool.tile([128, HIDDEN], F32, tag="ps",
                                              name="ps")
                for q in range(NQ):
                    for m in group:
                        for n in range(2):
                            for bp in range(2):
                                for j in range(2):
                                    B0 = 4 * q + 2 * bp
                                    nc.tensor.matmul(
                                        psums[m][:, n * 512:(n + 1) * 512],
                                        lhsT=xvs[m][:, B0:B0 + 2, :, j:j + 1],
                                        rhs=wvs[n][q][:, 2 * bp:2 * bp + 2,
                                                      :, j:j + 1],
                                        start=(q == 0 and bp == 0 and j == 0),
                                        stop=(q == NQ - 1 and bp == 1
                                              and j == 1),
                                        perf_mode=DR)
                for m in group:
                    epilogue(m, psums[m], inps[m])

    nc.compile()
    return nc


_last_results = None


def kernel(hidden_states, input_tensor, W, b, clip_val, gamma, beta):
    global _last_results
    hidden_states = np.asarray(hidden_states)
    input_tensor = np.asarray(input_tensor)
    W = np.asarray(W, dtype=np.float32)
    b = np.asarray(b, dtype=np.float32)
    gamma = np.asarray(gamma, dtype=np.float32)
    beta = np.asarray(beta, dtype=np.float32)
    clip = float(np.asarray(clip_val))

    use_b = bool(np.any(b != 0.0))
    use_gamma = bool(np.any(gamma != 1.0))
    use_beta = bool(np.any(beta != 0.0))
    scale_mul = abs(clip) / INTER

    key = (scale_mul, use_b, use_gamma, use_beta)
    if key not in _cache:
        _cache[key] = _build(scale_mul, use_b, use_gamma, use_beta)
    nc = _cache[key]

    hs = np.ascontiguousarray(
        hidden_states.reshape(TOKENS, INTER).astype(np.float32, copy=False))
    rs = np.ascontiguousarray(
        input_tensor.reshape(TOKENS, HIDDEN).astype(np.float32, copy=False))
    Wc = np.ascontiguousarray(W)

    in_maps = []
    for c in range(N_CORES):
        m = {
            "x": np.ascontiguousarray(hs[c * TPC:(c + 1) * TPC]),
            "res": np.ascontiguousarray(rs[c * TPC:(c + 1) * TPC]),
            "w": Wc,
        }
        if use_b:
            m["bvec"] = b
        if use_gamma:
            m["gvec"] = gamma
        if use_beta:
            m["bevec"] = beta
        in_maps.append(m)

    kwargs = {}
    if TRACE:
        _install_ntff_hook()
        kwargs["trace"] = True
        if TRACE_ALL_CORES:
            kwargs["trace_cores"] = list(range(N_CORES))
    res = bass_utils.run_bass_kernel_spmd(
        nc, in_maps, core_ids=list(range(N_CORES)), **kwargs)
    _last_results = res

    y = np.concatenate([res.results[c]["y"] for c in range(N_CORES)], axis=0)
    return y.reshape(hidden_states.shape[:-1] + (HIDDEN,)).astype(np.float32)
